# revision 43
# baseline (speedup 1.0000x reference)
"""CRF (dense projection + Viterbi decode) Trainium2 kernel.

Strategy (8 NeuronCores, data-parallel on batch):
  - Each core gets B_c = 32 sequences.
  - Phase 1: potentials = x @ W + b (+ boundary energies) on the PE.
    x is transposed on host so matmul lhsT tiles load contiguously.
  - Phase 2: Viterbi forward DP on the DVE. Partition layout packs
    (b, next-quarter) pairs into all 128 partitions: p = b*4 + ng, with
    each partition owning 12 "next" tags. Per step:
      scores[p, prev, j] = C[prev, ng*12+j] + alpha[b, prev]    (TT add)
      premax[p, j]       = max over prev                        (reduce)
      eq/iota-dot        -> backpointer (exact first-index argmax)
      alpha'             = premax + pot[t]                      (TT add)
      stream_shuffle x4  -> replicate alpha' quarters to all partitions
    Backpointers (as 47-p floats) stream to DRAM.
  - Backtrace runs on host from the returned backpointer planes.
"""

import numpy as np

import concourse.bass as bass
import concourse.bacc as bacc
import concourse.mybir as mybir
from concourse import bass_utils
from concourse.tile import TileContext

F32 = mybir.dt.float32
BF16 = mybir.dt.bfloat16
from concourse.bass import _add_dep_helper
ADD = mybir.AluOpType.add
MULT = mybir.AluOpType.mult
MAXOP = mybir.AluOpType.max
EQOP = mybir.AluOpType.is_equal
AXX = mybir.AxisListType.X

B, T, F, U = 256, 2048, 128, 48
NCORES = 8
BC = B // NCORES          # 32 sequences per core
NG = 4                    # "next"-tag quarters
J = U // NG               # 12 next tags per partition
P = 128                   # partitions

SHUF_MASKS = [[(i // NG) * NG + ng for i in range(32)] for ng in range(NG)]


def build_nc(t_steps=T, blk=128):
    nc = bacc.Bacc("TRN2", target_bir_lowering=False, debug=False,
                   enable_asserts=False)
    n_rows = BC * t_steps

    xT = nc.dram_tensor("xT", [F, n_rows], F32, kind="ExternalInput")
    w_in = nc.dram_tensor("w_in", [F, U], F32, kind="ExternalInput")
    bias_rep = nc.dram_tensor("bias_rep", [P, U], F32, kind="ExternalInput")
    lb32 = nc.dram_tensor("lb32", [BC, U], F32, kind="ExternalInput")
    rb32 = nc.dram_tensor("rb32", [BC, U], F32, kind="ExternalInput")
    c_rep = nc.dram_tensor("c_rep", [P, U * J], F32, kind="ExternalInput")
    iota_rev = nc.dram_tensor("iota_rev", [P, U * J], F32, kind="ExternalInput")

    pots_out = nc.dram_tensor("pots_out", [BC, t_steps, U], F32,
                              kind="ExternalOutput")
    bpk_out = nc.dram_tensor("bpk_out", [P, t_steps, J], F32,
                             kind="ExternalOutput")
    alphab_out = nc.dram_tensor("alphab_out", [P, U], F32,
                                kind="ExternalOutput")

    with TileContext(nc) as tc:
        with (
            tc.tile_pool(name="const", bufs=1) as cpool,
            tc.tile_pool(name="dramp", bufs=1, space="DRAM") as dpool,
        ):
            wt = cpool.tile([F, U], F32)
            nc.sync.dma_start(wt[:], w_in.ap())
            bt = cpool.tile([P, U], F32)
            nc.sync.dma_start(bt[:], bias_rep.ap())
            crt = cpool.tile([P, U * J], F32)
            nc.sync.dma_start(crt[:], c_rep.ap())
            irt = cpool.tile([P, U * J], F32)
            nc.sync.dma_start(irt[:], iota_rev.ap())
            lbt = cpool.tile([BC, U], F32)
            nc.sync.dma_start(lbt[:], lb32.ap())
            rbt = cpool.tile([BC, U], F32)
            nc.sync.dma_start(rbt[:], rb32.ap())
            alphaB = cpool.tile([P, U], F32)
            potsdp = dpool.tile([t_steps, BC, U], F32)

            # ---------------- Phase 1: potentials ----------------
            # The PE LDWEIGHTS struct encodes a single semaphore wait, so
            # each matmul may carry at most one. "Carrier" ldweights
            # instructions absorb the cross-engine waits (weight-DMA at
            # start, PSUM WAR vs the vector bias-add in steady state),
            # leaving each real matmul with only its input-DMA wait.
            tiles_per_b = t_steps // P
            n_tiles = BC * tiles_per_b
            with (
                tc.tile_pool(name="p1", bufs=4) as pool1,
                tc.tile_pool(name="ptp", bufs=n_tiles) as pool_pt,
                tc.tile_pool(name="p1ps", bufs=4, space="PSUM") as pspool,
                tc.tile_pool(name="p2io", bufs=2) as pool_io,
                tc.tile_pool(name="p2w", bufs=3) as pool_w,
            ):
                for m in range(n_tiles):
                    bi = m // tiles_per_b
                    t0 = (m % tiles_per_b) * P
                    lt = pool1.tile([F, P], F32, tag="lt")
                    nc.sync.dma_start(lt[:], xT.ap()[:, m * P:(m + 1) * P])
                    ps = pspool.tile([P, U], F32, tag="ps")
                    nc.tensor.matmul(ps[:], lhsT=lt[:], rhs=wt[:],
                                     start=True, stop=True)
                    pt = pool_pt.tile([P, U], F32, tag="pt")
                    nc.vector.tensor_tensor(pt[:], ps[:], bt[:], op=ADD)
                    nc.sync.dma_start(potsdp[t0:t0 + P, bi, :], pt[:])

                # boundary energies: in-place fixup of potsdp rows 0 / T-1
                for (trow, bnd) in ((0, lbt), (t_steps - 1, rbt)):
                    fx = pool1.tile([BC, U], F32, tag="fx")
                    nc.sync.dma_start(fx[:], potsdp[trow, :, :])
                    nc.vector.tensor_tensor(fx[:], fx[:], bnd[:], op=ADD)
                    nc.sync.dma_start(potsdp[trow, :, :], fx[:])

                # full potentials output: bulk relayout [t,b,u]->[b,t,u]
                # (chunked: per-DMA dim counts must fit 16-bit ISA fields)
                tchunk = min(512, t_steps)
                for tc0 in range(0, t_steps, tchunk):
                    nc.sync.dma_start(
                        pots_out.ap()[:, tc0:tc0 + tchunk, :],
                        potsdp[tc0:tc0 + tchunk, :, :].rearrange(
                            "t b u -> b t u"))

                # ---------------- Phase 2: Viterbi forward ----------------
                crt_v = crt[:].rearrange("p (u j) -> p u j", j=J)
                irt_v = irt[:].rearrange("p (u j) -> p u j", j=J)
                pdp_v = potsdp[:].rearrange("t b (ng j) -> (b ng) t j", ng=NG)
                n_blk = t_steps // blk
                # software pipeline: the backpointer extraction (eq/mk on
                # GPSIMD, final reduce on DVE one step late) runs off the
                # critical alpha chain.
                pending = None  # (mk_tile, bpkb_tile, tl)
                prev_bpkb = None

                def flush_pending():
                    nonlocal pending
                    if pending is not None:
                        mk_, bpkb_, tl_ = pending
                        nc.vector.tensor_reduce(
                            bpkb_[:, tl_, :], mk_[:].transpose([0, 2, 1]),
                            axis=AXX, op=MAXOP)
                        pending = None

                for kb in range(n_blk):
                    potb = pool_io.tile([P, blk, J], F32, tag="potb")
                    nc.sync.dma_start(
                        potb[:], pdp_v[:, kb * blk:(kb + 1) * blk, :])
                    bpkb = pool_io.tile([P, blk, J], F32, tag="bpkb")
                    for tl in range(blk):
                        t = kb * blk + tl
                        if t == 0:
                            for ng in range(NG):
                                nc.vector.stream_shuffle(
                                    alphaB[:, ng * J:(ng + 1) * J],
                                    potb[:, 0, :], mask=SHUF_MASKS[ng])
                            nc.vector.tensor_scalar_mul(
                                bpkb[:, 0, :], potb[:, 0, :], 0.0)
                            continue
                        sc = pool_w.tile([P, U, J], F32, tag="sc")
                        nc.vector.tensor_tensor(
                            sc[:], crt_v,
                            alphaB[:, :, None].to_broadcast([P, U, J]),
                            op=ADD)
                        pm = pool_w.tile([P, J], F32, tag="pm")
                        nc.vector.tensor_reduce(
                            pm[:], sc[:].transpose([0, 2, 1]),
                            axis=AXX, op=MAXOP)
                        eq = pool_w.tile([P, U, J], F32, tag="eq")
                        nc.vector.tensor_tensor(
                            eq[:], sc[:],
                            pm[:, None, :].to_broadcast([P, U, J]), op=EQOP)
                        mk = pool_w.tile([P, U, J], F32, tag="mk")
                        nc.vector.tensor_tensor(mk[:], eq[:], irt_v, op=MULT)
                        apn = pool_w.tile([P, J], F32, tag="apn")
                        nc.vector.tensor_tensor(
                            apn[:], pm[:], potb[:, tl, :], op=ADD)
                        for ng in range(NG):
                            nc.vector.stream_shuffle(
                                alphaB[:, ng * J:(ng + 1) * J], apn[:],
                                mask=SHUF_MASKS[ng])
                        flush_pending()
                        pending = (mk, bpkb, tl)
                    if prev_bpkb is not None:
                        nc.sync.dma_start(
                            bpk_out.ap()[:, (kb - 1) * blk:kb * blk, :],
                            prev_bpkb[:])
                    prev_bpkb = bpkb
                flush_pending()
                nc.sync.dma_start(
                    bpk_out.ap()[:, (n_blk - 1) * blk:n_blk * blk, :],
                    prev_bpkb[:])
            nc.sync.dma_start(alphab_out.ap(), alphaB[:])
    nc.compile()
    return nc


def _host_inputs(x, w, bvec, chain, lb, rb, t_steps=T):
    """Per-core input maps (host-side prep)."""
    x = np.ascontiguousarray(np.asarray(x, dtype=np.float32)[:, :t_steps, :])
    w = np.asarray(w, dtype=np.float32)
    chain = np.asarray(chain, dtype=np.float32)
    bias_rep = np.broadcast_to(np.asarray(bvec, np.float32), (P, U)).copy()
    lb32 = np.broadcast_to(np.asarray(lb, np.float32), (BC, U)).copy()
    rb32 = np.broadcast_to(np.asarray(rb, np.float32), (BC, U)).copy()
    # c_rep[p, prev*J + j] = chain[prev, (p%4)*J + j]
    ngidx = np.arange(P) % NG
    c_rep = np.empty((P, U * J), np.float32)
    for ng in range(NG):
        block = chain[:, ng * J:(ng + 1) * J].reshape(1, U * J)
        c_rep[ngidx == ng] = block
    iota_rev = np.broadcast_to(
        (47.0 - np.arange(U, dtype=np.float32))[:, None], (U, J)
    ).reshape(1, U * J)
    iota_rev = np.broadcast_to(iota_rev, (P, U * J)).copy()
    in_maps = []
    for c in range(NCORES):
        xc = x[c * BC:(c + 1) * BC]
        xT = np.ascontiguousarray(xc.reshape(BC * t_steps, F).T)
        in_maps.append({
            "xT": xT, "w_in": w, "bias_rep": bias_rep,
            "lb32": lb32, "rb32": rb32,
            "c_rep": c_rep, "iota_rev": iota_rev,
        })
    return in_maps


def _assemble(results, t_steps=T):
    """Combine per-core outputs; run host backtrace. Returns (tags, pots)."""
    pots = np.concatenate([r["pots_out"] for r in results], axis=0)
    nb = len(results) * BC
    # bp planes: bpk[p, t, j] with p = b*4+ng -> bp[t, b, u]
    bp = np.empty((t_steps, nb, U), np.int32)
    for c, r in enumerate(results):
        k = r["bpk_out"].reshape(BC, NG, t_steps, J)      # [b, ng, t, j]
        v = 47.0 - np.transpose(k, (2, 0, 1, 3)).reshape(t_steps, BC, U)
        bp[:, c * BC:(c + 1) * BC, :] = np.rint(v).astype(np.int32)
    alpha_fin = np.stack(
        [r["alphab_out"][np.arange(BC) * NG] for r in results]
    ).reshape(nb, U)
    tags = np.empty((t_steps, nb), np.int32)
    tags[t_steps - 1] = np.argmax(alpha_fin, axis=1)
    for t in range(t_steps - 2, -1, -1):
        tags[t] = np.take_along_axis(
            bp[t + 1], tags[t + 1][:, None], axis=1)[:, 0]
    return np.ascontiguousarray(tags.T), pots


_NC_CACHE = {}
LAST_EXEC_NS = None


def kernel(x, W, b, chain_kernel, left_boundary, right_boundary):
    global LAST_EXEC_NS
    import os
    t_steps = T
    key = t_steps
    if key not in _NC_CACHE:
        _NC_CACHE[key] = build_nc(t_steps)
    nc = _NC_CACHE[key]
    in_maps = _host_inputs(x, W, b, chain_kernel, left_boundary,
                           right_boundary, t_steps)
    trace = os.environ.get("VIT_TRACE", "0") == "1"
    try:
        res = bass_utils.run_bass_kernel_spmd(nc, in_maps,
                                              core_ids=list(range(NCORES)),
                                              trace=trace)
    except ModuleNotFoundError:
        res = bass_utils.run_bass_kernel_spmd(nc, in_maps,
                                              core_ids=list(range(NCORES)))
    LAST_EXEC_NS = res.exec_time_ns
    tags, pots = _assemble(res.results, t_steps)
    seq_len = np.full((tags.shape[0],), t_steps, dtype=np.int32)
    chain = np.asarray(chain_kernel, dtype=np.float32)
    return tags, pots, seq_len, chain
